# revision 7
# baseline (speedup 1.0000x reference)
"""Trainium2 Bass kernel for the B-spline (KAN-style) layer.

Math: out[b,o] = sum_{i,k} B3_k(t(b,i)) * coeff[i,o,k], where t = tanh(x)
mapped to knot coordinates t = (tanh(x) - grid[0]) / h in (3, 8), and B3 are
cubic B-spline bases over uniform integer knots.

Key transformation: each cubic B-spline basis is an exact linear combination
of truncated cubes L_m = relu(t-m)^3 and R_m = relu(m-t)^3, m in {4,5,6,7}
(divided-difference / truncated-power identity, binomial weights [1,-4,6,-4,1]/6).
The 8x8 basis-change matrix is folded into the coefficient tensor on the host,
so the device only computes 8 "plane" tensors per input feature:
    w_m = 2.5*tanh(x) + (5.5 - m)            (affine)
    s_m = w_m^2                              (ScalarE Square)
    L_m = max(w_m,0)*s_m ; Rn_m = min(w_m,0)*s_m   (fused scalar_tensor_tensor)
(Rn_m = -R_m; sign folded into the host coefficients.)

Then a dense fp16 matmul: out[o,b] = sum_{(i,plane)} C3[(i,plane),o] * rho[(i,plane),b]
with contraction K = 512*8 = 4096, run on the PE at full bf16/fp16 rate.

Sharding: data-parallel over batch (8192 -> 8 x 1024); coefficients replicated.
Inputs are transposed on the host so the feature dim i lands on SBUF partitions
(making the contraction dim the partition dim for the matmul with no on-device
transposes); the output is produced as (o, b) and transposed back on the host.
"""

from contextlib import ExitStack

import numpy as np

import concourse.bass as bass
import concourse.mybir as mybir
import concourse.tile as tile
from concourse.bass_utils import run_bass_kernel_spmd
from concourse.vector_clock import ScopedClock

F32 = mybir.dt.float32
F16 = mybir.dt.float16

N_CORES = 8
B_FULL = 8192
B_SHARD = B_FULL // N_CORES  # 1024
I_FEAT = 512
O_FEAT = 512
NPLANES = 8
NCHUNK = I_FEAT // 128  # 4
ALU = mybir.AluOpType
AF = mybir.ActivationFunctionType

# ---------------------------------------------------------------------------
# Workaround for walrus "Too many sync wait commands" on the TileContext final
# Drain: spread the accumulated semaphore waits across single-wait nofuse NOPs
# on the sync engine, then emit a bare drain + the usual barrier/cleanup.
_MAXW = 1


def _patched_drain_and_barrier(self, tick_clock, wait_clock):
    nc = self.nc
    probe = nc.sync.nop(nofuse=True)
    wait_clock.add_sem_waits(probe.ins, ScopedClock({None: tick_clock.global_clock}))
    si = probe.ins.sync_info
    waits = list(si.on_wait) if si and si.on_wait else []
    if len(waits) > _MAXW:
        si.on_wait = waits[:_MAXW]
        rest = waits[_MAXW:]
        while rest:
            chunk, rest = rest[:_MAXW], rest[_MAXW:]
            n2 = nc.sync.nop(nofuse=True)
            s2 = n2.ins.sync_info
            if s2 is None:
                n2.ins.sync_info = mybir.SyncInfo(on_wait=chunk, on_update=[])
            else:
                s2.on_wait = chunk
    nc.sync.drain()
    nc.all_engine_barrier()
    assert self.sems is not None
    popped = nc._tile_sem_poison_stack.pop()
    assert popped is self._sem_poison
    nc.clear_and_free_semaphores(list(self.sems.allocated().values()))
    nc.all_engine_barrier()


tile.TileContext._drain_and_barrier = _patched_drain_and_barrier


def _split_all_waits(nc: bass.Bass) -> None:
    """This image's walrus rejects instructions carrying more than one sync
    wait. Hoist all but the last wait of each instruction onto fresh NoOps on
    the same engine immediately before it (in-order issue makes this
    equivalent, merely slightly stronger synchronization)."""
    cnt = 0
    for f in nc.m.functions:
        for bb in f.blocks:
            out = []
            changed = False
            for inst in bb.instructions:
                si = inst.sync_info
                waits = list(si.on_wait) if si and si.on_wait else []
                if len(waits) > 1:
                    changed = True
                    for w in waits[:-1]:
                        nop = mybir.InstNoOp(name=f"waitsplit-{cnt}", ins=[], outs=[])
                        cnt += 1
                        nop.engine = inst.engine
                        nop.sync_info = mybir.SyncInfo(on_wait=[w], on_update=[])
                        out.append(nop)
                    si.on_wait = [waits[-1]]
                out.append(inst)
            if changed:
                bb.instructions = out


# ---------------------------------------------------------------------------


def _build_nc(t_scale: float, t_bias: float) -> bass.Bass:
    """Build the per-core Bass program.

    Per-core I/O:
      xt : (512, 1024) f32   x^T shard (feature-major)
      c3 : (4, 128, 4096) f16  folded coefficients [chunk, part, plane*512+o]
      out: (512, 1024) f32   output (o, b) shard
    """
    nc = bass.Bass()
    xt = nc.declare_dram_parameter("xt", [I_FEAT, B_SHARD], F32, isOutput=False)
    c3 = nc.declare_dram_parameter(
        "c3", [NCHUNK, 128, NPLANES * O_FEAT], F16, isOutput=False
    )
    out = nc.declare_dram_parameter("out", [O_FEAT, B_SHARD], F32, isOutput=True)

    with tile.TileContext(nc) as tc, ExitStack() as ctx:
        c3_pool = ctx.enter_context(tc.tile_pool(name="c3", bufs=1))
        xin_pool = ctx.enter_context(tc.tile_pool(name="xin", bufs=2))
        xn_pool = ctx.enter_context(tc.tile_pool(name="xn", bufs=2))
        sq_pool = ctx.enter_context(tc.tile_pool(name="sq", bufs=3))
        aff_pool = ctx.enter_context(tc.tile_pool(name="aff", bufs=3))
        rho_pool = ctx.enter_context(tc.tile_pool(name="rho", bufs=1))
        ps_pool = ctx.enter_context(
            tc.tile_pool(name="ps", bufs=1, space=bass.MemorySpace.PSUM)
        )
        ost_pool = ctx.enter_context(tc.tile_pool(name="ost", bufs=1))

        # Coefficient chunks (one DMA per 1 MiB chunk).
        c3_sb = []
        for c in range(NCHUNK):
            t = c3_pool.tile([128, NPLANES * O_FEAT], F16, tag=f"c3_{c}")
            nc.sync.dma_start(t[:], c3[c])
            c3_sb.append(t)

        # Elementwise plane production: per chunk c, 8 planes rho[c][r]
        # (r=0..3 -> L_{4+r}, r=4..7 -> -R_{r} with sign folded into c3).
        rho = [[None] * NPLANES for _ in range(NCHUNK)]
        for c in range(NCHUNK):
            xt_sb = xin_pool.tile([128, B_SHARD], F32, tag="xt")
            nc.sync.dma_start(xt_sb[:], xt[c * 128 : (c + 1) * 128, :])
            xn = xn_pool.tile([128, B_SHARD], F16, tag="xn")
            nc.scalar.activation(xn[:], xt_sb[:], AF.Tanh)
            for mi, m in enumerate((4, 5, 6, 7)):
                beta = t_bias - m
                a = aff_pool.tile([128, B_SHARD], F16, tag="a")
                nc.vector.tensor_scalar(a[:], xn[:], t_scale, beta, ALU.mult, ALU.add)
                s = sq_pool.tile([128, B_SHARD], F16, tag="s")
                nc.scalar.activation(s[:], a[:], AF.Square)
                lp = rho_pool.tile([128, B_SHARD], F16, tag=f"rho{c}_{mi}")
                nc.vector.scalar_tensor_tensor(lp[:], a[:], 0.0, s[:], ALU.max, ALU.mult)
                rp = rho_pool.tile([128, B_SHARD], F16, tag=f"rho{c}_{mi + 4}")
                nc.vector.scalar_tensor_tensor(rp[:], a[:], 0.0, s[:], ALU.min, ALU.mult)
                rho[c][mi] = lp
                rho[c][mi + 4] = rp

        # Dense matmul: 8 PSUM tiles (o_chunk x b_half) accumulated over all
        # 32 (chunk, plane) K-slices; K-major loop keeps the PE dense and
        # consumes rho planes in production order.
        ps = [
            [
                ps_pool.tile(
                    [128, 512], F32, tag=f"ps{o}_{bh}", name=f"ps{o}_{bh}"
                )
                for bh in range(2)
            ]
            for o in range(NCHUNK)
        ]
        NK = NCHUNK * NPLANES
        for kk in range(NK):
            c, r = divmod(kk, NPLANES)
            rt = rho[c][r]
            for o in range(4):
                lhsT = c3_sb[c][:, r * O_FEAT + o * 128 : r * O_FEAT + (o + 1) * 128]
                for bh in range(2):
                    nc.tensor.matmul(
                        ps[o][bh][:],
                        lhsT,
                        rt[:, bh * 512 : (bh + 1) * 512],
                        start=(kk == 0),
                        stop=(kk == NK - 1),
                    )

        # PSUM -> SBUF -> DRAM.
        for o in range(4):
            for bh in range(2):
                ot = ost_pool.tile([128, 512], F32, tag=f"ot{o}_{bh}")
                if bh == 0:
                    nc.scalar.activation(ot[:], ps[o][bh][:], AF.Copy)
                else:
                    nc.vector.tensor_copy(ot[:], ps[o][bh][:])
                nc.sync.dma_start(
                    out[o * 128 : (o + 1) * 128, bh * 512 : (bh + 1) * 512], ot[:]
                )
    _split_all_waits(nc)
    return nc


# Basis-change: B3[j](t) = sum_r W[j,r] * plane_r(t), planes ordered
# [L4,L5,L6,L7, R4,R5,R6,R7]; binomial divided-difference weights /6.
_W6 = np.array(
    [
        [0, 0, 0, 0, 1, 0, 0, 0],
        [0, 0, 0, 0, -4, 1, 0, 0],
        [0, 0, 0, 0, 6, -4, 1, 0],
        [0, 0, 0, 0, -4, 6, -4, 1],
        [1, -4, 6, -4, 0, 0, 0, 0],
        [0, 1, -4, 6, 0, 0, 0, 0],
        [0, 0, 1, -4, 0, 0, 0, 0],
        [0, 0, 0, 1, 0, 0, 0, 0],
    ],
    dtype=np.float64,
)

_nc_cache: dict = {}


def _prepare(x: np.ndarray, coefficients: np.ndarray, grid: np.ndarray):
    x = np.asarray(x, dtype=np.float32)
    coefficients = np.asarray(coefficients, dtype=np.float32)
    grid = np.asarray(grid, dtype=np.float32)

    # Knot-coordinate transform t = (tanh(x) - grid[0]) / h (uniform grid).
    h = float(grid[-1] - grid[0]) / (len(grid) - 1)
    t_scale = 1.0 / h
    t_bias = -float(grid[0]) / h  # t = t_scale * xn + t_bias; here 2.5, 5.5

    key = (round(t_scale, 9), round(t_bias, 9))
    if key not in _nc_cache:
        _nc_cache[key] = _build_nc(t_scale, t_bias)
    nc = _nc_cache[key]

    # Host-side coefficient fold: C3[i, r, o] = sum_j coeff[i,o,j] * W[j,r] / 6,
    # with R-plane columns negated (device computes -R via min(w,0)*w^2).
    w = _W6 / 6.0
    w[:, 4:] *= -1.0
    c3f = np.einsum("ioj,jr->iro", coefficients.astype(np.float64), w)
    c3_arr = np.ascontiguousarray(
        c3f.reshape(NCHUNK, 128, NPLANES, O_FEAT)
        .reshape(NCHUNK, 128, NPLANES * O_FEAT)
        .astype(np.float16)
    )

    xt = np.ascontiguousarray(x.T)  # (512, 8192)
    in_maps = [
        {
            "xt": np.ascontiguousarray(xt[:, c * B_SHARD : (c + 1) * B_SHARD]),
            "c3": c3_arr,
        }
        for c in range(N_CORES)
    ]
    return nc, in_maps


def kernel(x: np.ndarray, coefficients: np.ndarray, grid: np.ndarray) -> np.ndarray:
    nc, in_maps = _prepare(x, coefficients, grid)
    res = run_bass_kernel_spmd(nc, in_maps, list(range(N_CORES)), trace=False)
    out_t = np.concatenate(
        [res.results[i]["out"] for i in range(N_CORES)], axis=1
    )  # (512, 8192)
    return np.ascontiguousarray(out_t.T).astype(np.float32)


# revision 8
# speedup vs baseline: 1.1797x; 1.1797x over previous
"""Trainium2 Bass kernel for the B-spline (KAN-style) layer.

Math: out[b,o] = sum_{i,k} B3_k(t(b,i)) * coeff[i,o,k], where t = tanh(x)
mapped to knot coordinates t = (tanh(x) - grid[0]) / h in (3, 8), and B3 are
cubic B-spline bases over uniform integer knots.

Key transformation: each cubic B-spline basis is an exact linear combination
of truncated cubes L_m = relu(t-m)^3 and R_m = relu(m-t)^3, m in {4,5,6,7}
(divided-difference / truncated-power identity, binomial weights [1,-4,6,-4,1]/6).
The 8x8 basis-change matrix is folded into the coefficient tensor on the host,
so the device only computes 8 "plane" tensors per input feature:
    w_m = 2.5*tanh(x) + (5.5 - m)            (affine)
    s_m = w_m^2                              (ScalarE Square)
    L_m = max(w_m,0)*s_m ; Rn_m = min(w_m,0)*s_m   (fused scalar_tensor_tensor)
(Rn_m = -R_m; sign folded into the host coefficients.)

Then a dense fp16 matmul: out[o,b] = sum_{(i,plane)} C3[(i,plane),o] * rho[(i,plane),b]
with contraction K = 512*8 = 4096, run on the PE at full bf16/fp16 rate.

Sharding: data-parallel over batch (8192 -> 8 x 1024); coefficients replicated.
Inputs are transposed on the host so the feature dim i lands on SBUF partitions
(making the contraction dim the partition dim for the matmul with no on-device
transposes); the output is produced as (o, b) and transposed back on the host.
"""

from contextlib import ExitStack

import numpy as np

import concourse.bass as bass
import concourse.mybir as mybir
import concourse.tile as tile
from concourse.bass_utils import run_bass_kernel_spmd
from concourse.vector_clock import ScopedClock

F32 = mybir.dt.float32
F16 = mybir.dt.float16

N_CORES = 8
B_FULL = 8192
B_SHARD = B_FULL // N_CORES  # 1024
I_FEAT = 512
O_FEAT = 512
NPLANES = 8
NCHUNK = I_FEAT // 128  # 4
ALU = mybir.AluOpType
AF = mybir.ActivationFunctionType

# ---------------------------------------------------------------------------
# Workaround for walrus "Too many sync wait commands" on the TileContext final
# Drain: spread the accumulated semaphore waits across single-wait nofuse NOPs
# on the sync engine, then emit a bare drain + the usual barrier/cleanup.
_MAXW = 1


def _patched_drain_and_barrier(self, tick_clock, wait_clock):
    nc = self.nc
    probe = nc.sync.nop(nofuse=True)
    wait_clock.add_sem_waits(probe.ins, ScopedClock({None: tick_clock.global_clock}))
    si = probe.ins.sync_info
    waits = list(si.on_wait) if si and si.on_wait else []
    if len(waits) > _MAXW:
        si.on_wait = waits[:_MAXW]
        rest = waits[_MAXW:]
        while rest:
            chunk, rest = rest[:_MAXW], rest[_MAXW:]
            n2 = nc.sync.nop(nofuse=True)
            s2 = n2.ins.sync_info
            if s2 is None:
                n2.ins.sync_info = mybir.SyncInfo(on_wait=chunk, on_update=[])
            else:
                s2.on_wait = chunk
    nc.sync.drain()
    nc.all_engine_barrier()
    assert self.sems is not None
    popped = nc._tile_sem_poison_stack.pop()
    assert popped is self._sem_poison
    nc.clear_and_free_semaphores(list(self.sems.allocated().values()))
    nc.all_engine_barrier()


tile.TileContext._drain_and_barrier = _patched_drain_and_barrier


def _split_all_waits(nc: bass.Bass) -> None:
    """This image's walrus rejects instructions carrying more than one sync
    wait. Hoist all but the last wait of each instruction onto fresh NoOps on
    the same engine immediately before it (in-order issue makes this
    equivalent, merely slightly stronger synchronization)."""
    cnt = 0
    for f in nc.m.functions:
        for bb in f.blocks:
            out = []
            changed = False
            for inst in bb.instructions:
                si = inst.sync_info
                waits = list(si.on_wait) if si and si.on_wait else []
                if len(waits) > 1:
                    changed = True
                    for w in waits[:-1]:
                        nop = mybir.InstNoOp(name=f"waitsplit-{cnt}", ins=[], outs=[])
                        cnt += 1
                        nop.engine = inst.engine
                        nop.sync_info = mybir.SyncInfo(on_wait=[w], on_update=[])
                        out.append(nop)
                    si.on_wait = [waits[-1]]
                out.append(inst)
            if changed:
                bb.instructions = out


# ---------------------------------------------------------------------------


def _build_nc(t_scale: float, t_bias: float) -> bass.Bass:
    """Build the per-core Bass program.

    Per-core I/O:
      xt : (512, 1024) f32   x^T shard (feature-major)
      c3 : (4, 128, 4096) f16  folded coefficients [chunk, part, plane*512+o]
      out: (512, 1024) f32   output (o, b) shard
    """
    nc = bass.Bass()
    xt = nc.declare_dram_parameter("xt", [I_FEAT, B_SHARD], F32, isOutput=False)
    c3 = nc.declare_dram_parameter(
        "c3", [NCHUNK, 128, NPLANES * O_FEAT], F16, isOutput=False
    )
    out = nc.declare_dram_parameter("out", [O_FEAT, B_SHARD], F32, isOutput=True)

    with tile.TileContext(nc) as tc, ExitStack() as ctx:
        c3_pool = ctx.enter_context(tc.tile_pool(name="c3", bufs=1))
        xin_pool = ctx.enter_context(tc.tile_pool(name="xin", bufs=2))
        xn_pool = ctx.enter_context(tc.tile_pool(name="xn", bufs=2))
        sq_pool = ctx.enter_context(tc.tile_pool(name="sq", bufs=3))
        aff_pool = ctx.enter_context(tc.tile_pool(name="aff", bufs=3))
        rho_pool = ctx.enter_context(tc.tile_pool(name="rho", bufs=1))
        ps_pool = ctx.enter_context(
            tc.tile_pool(name="ps", bufs=1, space=bass.MemorySpace.PSUM)
        )
        ost_pool = ctx.enter_context(tc.tile_pool(name="ost", bufs=1))

        # Input DMAs interleaved per chunk (xt[c] gates the tanh->plane chain,
        # c3[c] gates the matmuls) so compute starts as early as possible.
        # Elementwise plane production: per chunk c, 8 planes rho[c][r]
        # (r=0..3 -> L_{4+r}, r=4..7 -> -R_{r} with sign folded into c3).
        c3_sb = []
        rho = [[None] * NPLANES for _ in range(NCHUNK)]
        for c in range(NCHUNK):
            xt_sb = xin_pool.tile([128, B_SHARD], F32, tag="xt")
            nc.sync.dma_start(xt_sb[:], xt[c * 128 : (c + 1) * 128, :])
            ct = c3_pool.tile([128, NPLANES * O_FEAT], F16, tag=f"c3_{c}")
            nc.sync.dma_start(ct[:], c3[c])
            c3_sb.append(ct)
            xn = xn_pool.tile([128, B_SHARD], F16, tag="xn")
            nc.scalar.activation(xn[:], xt_sb[:], AF.Tanh)
            for mi, m in enumerate((4, 5, 6, 7)):
                beta = t_bias - m
                a = aff_pool.tile([128, B_SHARD], F16, tag="a")
                nc.vector.tensor_scalar(a[:], xn[:], t_scale, beta, ALU.mult, ALU.add)
                s = sq_pool.tile([128, B_SHARD], F16, tag="s")
                nc.scalar.activation(s[:], a[:], AF.Square)
                cc = sq_pool.tile([128, B_SHARD], F16, tag="cube")
                nc.vector.tensor_mul(cc[:], s[:], a[:])
                lp = rho_pool.tile([128, B_SHARD], F16, tag=f"rho{c}_{mi}")
                nc.vector.tensor_scalar_max(lp[:], cc[:], 0.0)
                rp = rho_pool.tile([128, B_SHARD], F16, tag=f"rho{c}_{mi + 4}")
                nc.vector.tensor_scalar_min(rp[:], cc[:], 0.0)
                rho[c][mi] = lp
                rho[c][mi + 4] = rp

        # Dense matmul: 8 PSUM tiles (o_chunk x b_half) accumulated over all
        # 32 (chunk, plane) K-slices; K-major loop keeps the PE dense and
        # consumes rho planes in production order. b-half-major so the first
        # half's PSUM eviction + output DMA overlap the second half's matmuls.
        ps = [
            [
                ps_pool.tile(
                    [128, 512], F32, tag=f"ps{o}_{bh}", name=f"ps{o}_{bh}"
                )
                for bh in range(2)
            ]
            for o in range(NCHUNK)
        ]
        NK = NCHUNK * NPLANES
        for bh in range(2):
            for kk in range(NK):
                c, r = divmod(kk, NPLANES)
                rt = rho[c][r]
                for o in range(4):
                    lhsT = c3_sb[c][
                        :, r * O_FEAT + o * 128 : r * O_FEAT + (o + 1) * 128
                    ]
                    nc.tensor.matmul(
                        ps[o][bh][:],
                        lhsT,
                        rt[:, bh * 512 : (bh + 1) * 512],
                        start=(kk == 0),
                        stop=(kk == NK - 1),
                    )
            # PSUM -> SBUF -> DRAM for this half; overlaps the next half's
            # matmuls (split between ACT and DVE).
            for o in range(4):
                ot = ost_pool.tile([128, 512], F32, tag=f"ot{o}_{bh}")
                if o % 2 == 0:
                    nc.scalar.activation(ot[:], ps[o][bh][:], AF.Copy)
                else:
                    nc.vector.tensor_copy(ot[:], ps[o][bh][:])
                nc.sync.dma_start(
                    out[o * 128 : (o + 1) * 128, bh * 512 : (bh + 1) * 512], ot[:]
                )
    _split_all_waits(nc)
    return nc


# Basis-change: B3[j](t) = sum_r W[j,r] * plane_r(t), planes ordered
# [L4,L5,L6,L7, R4,R5,R6,R7]; binomial divided-difference weights /6.
_W6 = np.array(
    [
        [0, 0, 0, 0, 1, 0, 0, 0],
        [0, 0, 0, 0, -4, 1, 0, 0],
        [0, 0, 0, 0, 6, -4, 1, 0],
        [0, 0, 0, 0, -4, 6, -4, 1],
        [1, -4, 6, -4, 0, 0, 0, 0],
        [0, 1, -4, 6, 0, 0, 0, 0],
        [0, 0, 1, -4, 0, 0, 0, 0],
        [0, 0, 0, 1, 0, 0, 0, 0],
    ],
    dtype=np.float64,
)

_nc_cache: dict = {}


def _prepare(x: np.ndarray, coefficients: np.ndarray, grid: np.ndarray):
    x = np.asarray(x, dtype=np.float32)
    coefficients = np.asarray(coefficients, dtype=np.float32)
    grid = np.asarray(grid, dtype=np.float32)

    # Knot-coordinate transform t = (tanh(x) - grid[0]) / h (uniform grid).
    h = float(grid[-1] - grid[0]) / (len(grid) - 1)
    t_scale = 1.0 / h
    t_bias = -float(grid[0]) / h  # t = t_scale * xn + t_bias; here 2.5, 5.5

    key = (round(t_scale, 9), round(t_bias, 9))
    if key not in _nc_cache:
        _nc_cache[key] = _build_nc(t_scale, t_bias)
    nc = _nc_cache[key]

    # Host-side coefficient fold: C3[i, r, o] = sum_j coeff[i,o,j] * W[j,r] / 6,
    # with R-plane columns negated (device computes -R via min(w,0)*w^2).
    w = _W6 / 6.0
    w[:, 4:] *= -1.0
    c3f = np.einsum("ioj,jr->iro", coefficients.astype(np.float64), w)
    c3_arr = np.ascontiguousarray(
        c3f.reshape(NCHUNK, 128, NPLANES, O_FEAT)
        .reshape(NCHUNK, 128, NPLANES * O_FEAT)
        .astype(np.float16)
    )

    xt = np.ascontiguousarray(x.T)  # (512, 8192)
    in_maps = [
        {
            "xt": np.ascontiguousarray(xt[:, c * B_SHARD : (c + 1) * B_SHARD]),
            "c3": c3_arr,
        }
        for c in range(N_CORES)
    ]
    return nc, in_maps


def kernel(x: np.ndarray, coefficients: np.ndarray, grid: np.ndarray) -> np.ndarray:
    nc, in_maps = _prepare(x, coefficients, grid)
    res = run_bass_kernel_spmd(nc, in_maps, list(range(N_CORES)), trace=False)
    out_t = np.concatenate(
        [res.results[i]["out"] for i in range(N_CORES)], axis=1
    )  # (512, 8192)
    return np.ascontiguousarray(out_t.T).astype(np.float32)
